# revision 28
# baseline (speedup 1.0000x reference)
"""CausalShapedAttention Trainium2 Bass kernel.

Problem: y = (beta*softmax(causal(q k^T / sqrt(D))) + alpha*I - gamma*MC) @ v
  with qk = x @ w_attn^T (q,k halves), v = x reshaped; B=2, T=2048, C=1024, H=16, D=64.
  MC[i,j] = 1/(T-1-i) for j>i (i<T-1); MC[T-1,:] = 1/T.

Sharding: 8 cores; core c -> batch b=c//4, head-group g=c%4 (4 heads each).
Each core is fully independent (no collectives). The host passes x^T and the
per-core W-slice^T so no on-chip transposes of the inputs are needed.

Per-core dataflow (transposed-S formulation, no attention-matrix transposes):
  qT,kT = W^T-chunks^T @ xT-chunks (PSUM accum)          [64, T] per head
  per head, per 512-wide query chunk ct:
    for key-block bj (128 rows of k, causal bj*128 <= chunk end):
      ST[j,i] = kT_bj . qT-chunk   (PE, fp32r)
      exp via ACT (fused 1/sqrt(D) scale) -> SBUF fp32r
      yTc[0:65, i] += [v_bj | 1]^T @ expST   (row 64 = softmax sums)
    normalize: yTout-chunk = yTc[0:64] * (beta/sums)  (ones-matmul broadcast)
    mc-chunk: dense column-constant-tile matmuls add -gamma*MC@v + alpha*v
  the dense last row of MC is patched analytically from mc[:,0] (closed form);
  yTout is PE-transposed back to natural [T, D] layout and DMA'd out.
"""
import sys

for _p in ("/opt/trn_rl_repo",):
    if _p not in sys.path:
        sys.path.insert(0, _p)

from contextlib import ExitStack

import numpy as np

import concourse.bass as bass
import concourse.tile as tile
from concourse import bacc, mybir
from concourse.bass_utils import run_bass_kernel_spmd

F32 = mybir.dt.float32
F32R = mybir.dt.float32r
EXP = mybir.ActivationFunctionType.Exp
OP = mybir.AluOpType

B, T, C, H, D = 2, 2048, 1024, 16, 64
HL = 4            # heads per core
GC = HL * D       # channels per head-group (256)
NCORES = 8
NB = T // 128     # 16 key/query row blocks
KC = C // 128     # 8 contraction chunks

# matmul dtype knobs (float32r = full-rate PE mode, fp32 = exact but 4 cyc/row)
CFG = dict(proj_r=True, st_r=True, pv_r=True, bc_r=True, mc_r=True)

LAST_RESULTS = None  # BassKernelResults of the most recent run (for test.py)


def _emit(tc: tile.TileContext, xt, xv, wt, y, alpha, beta, gamma, cfg):
    nc = tc.nc

    def r(ap, knob):
        return ap.bitcast(F32R) if cfg[knob] else ap.bitcast(F32)

    have_mc = gamma != 0.0
    have_ai = alpha != 0.0

    with ExitStack() as ctx:
        ctx.enter_context(nc.allow_low_precision(
            reason="float32r operands for full-rate PE matmuls"))
        consts = ctx.enter_context(tc.tile_pool(name="consts", bufs=1))

        # identity for PE transposes
        ident = consts.tile([128, 128], F32, name="ident", tag="ident")
        nc.vector.memset(ident, 1.0)
        nc.gpsimd.affine_select(
            out=ident, in_=ident, compare_op=OP.is_equal, fill=0.0,
            base=0, pattern=[[-1, 128]], channel_multiplier=1,
        )

        # beta row for the sums-broadcast matmul (rounded to fp32r)
        brow_f = consts.tile([1, 64], F32, name="brow_f", tag="brow_f")
        nc.vector.memset(brow_f, beta)
        brow = consts.tile([1, 64], F32R, name="brow", tag="brow")
        nc.vector.tensor_copy(out=brow, in_=brow_f)

        # additive causal mask for diagonal ST blocks: -1e30 where j > i
        negmask = consts.tile([128, 128], F32, name="negmask", tag="negmask")
        nc.vector.memset(negmask, -1e30)
        nc.gpsimd.affine_select(
            out=negmask, in_=negmask, compare_op=OP.is_gt, fill=0.0,
            base=0, pattern=[[-1, 128]], channel_multiplier=1,
        )
        ones_col = consts.tile([128, 1], F32, name="ones_col", tag="ones_col")
        nc.vector.memset(ones_col, 1.0)

        # v[0, :] per head as a [64,1] column (for the dense-last-row patch)
        v0col = [consts.tile([64, 1], F32, name=f"v0c{h}", tag=f"v0c{h}")
                 for h in range(HL)]
        for h in range(HL):
            nc.sync.dma_start(
                out=v0col[h],
                in_=xv[0:1, h * 64:(h + 1) * 64].rearrange("a b -> b a"))

        # persistent SBUF tensors
        qkT = [consts.tile([128, T], F32R, name=f"qkT{mt}", tag=f"qkT{mt}")
               for mt in range(4)]
        vtiles = [consts.tile([128, GC], F32, name=f"v{bt}", tag=f"v{bt}")
                  for bt in range(NB)]

        def pcopy(dst, src, i):
            # alternate PSUM->SBUF copies across DVE/ACT
            if i % 2 == 0:
                nc.vector.tensor_copy(out=dst, in_=src)
            else:
                nc.scalar.copy(out=dst, in_=src)

        # ---- mc / alpha*I constant tiles (DVE/Pool work, fills DMA window) ----
        mcb = None
        mcdiag = []
        with ExitStack() as s2:
            scratch = s2.enter_context(tc.tile_pool(name="mcscratch", bufs=1))
            aI_f = None
            if have_ai:
                aI_f = scratch.tile([128, 128], F32, name="aI_f", tag="aI_f")
                nc.vector.memset(aI_f, alpha)
                nc.gpsimd.affine_select(
                    out=aI_f, in_=aI_f, compare_op=OP.is_equal, fill=0.0,
                    base=0, pattern=[[-1, 128]], channel_multiplier=1,
                )
            if have_mc:
                mcs = scratch.tile([128, T], F32, name="mcs", tag="mcs")
                nc.gpsimd.iota(mcs, pattern=[[-1, T]], base=T - 1,
                               channel_multiplier=0,
                               allow_small_or_imprecise_dtypes=True)
                nc.vector.memset(mcs[:, T - 1:T], 1.0)  # avoid 1/0; fixed below
                mcf = scratch.tile([128, T], F32, name="mcf", tag="mcf")
                nc.vector.reciprocal(out=mcf, in_=mcs)
                nc.vector.tensor_scalar_mul(mcf, mcf, -gamma)
                nc.vector.memset(mcf[:, T - 1:T], 0.0)
                mcb = consts.tile([128, T], F32R, name="mcb", tag="mcb")
                nc.vector.tensor_copy(out=mcb, in_=mcf)
                mdf = scratch.tile([128, 128], F32, name="mdf", tag="mdf")
                for bj in range(NB):
                    # keep strictly-lower (j > i) of the column-constant strip
                    nc.gpsimd.affine_select(
                        out=mdf, in_=mcf[:, bj * 128:(bj + 1) * 128],
                        compare_op=OP.is_gt, fill=0.0,
                        base=0, pattern=[[-1, 128]], channel_multiplier=1,
                    )
                    if have_ai:
                        nc.vector.tensor_add(mdf, mdf, aI_f)
                    md = consts.tile([128, 128], F32R, name=f"mcd{bj}",
                                     tag=f"mcd{bj}")
                    nc.vector.tensor_copy(out=md, in_=mdf)
                    mcdiag.append(md)
            elif have_ai:
                aI = consts.tile([128, 128], F32R, name="aI", tag="aI")
                nc.vector.tensor_copy(out=aI, in_=aI_f)
                mcdiag = [aI] * NB

        # ---- PSUM pools (2+2+1+1+2 = 8 banks; proj shares the st slots) ----
        stp = ctx.enter_context(tc.tile_pool(name="stp", bufs=3, space="PSUM"))
        accp = ctx.enter_context(tc.tile_pool(name="accp", bufs=2, space="PSUM"))
        mccp = stp  # mcc tag shares the stp pool (1 extra bank)
        bcp = ctx.enter_context(tc.tile_pool(name="bcp", bufs=1, space="PSUM"))
        otp = ctx.enter_context(tc.tile_pool(name="otp", bufs=1, space="PSUM"))
        vbp = ctx.enter_context(tc.tile_pool(name="vbp", bufs=1))
        mcstage = ctx.enter_context(tc.tile_pool(name="mcstage", bufs=1))
        late = {}  # attention-phase SBUF pools, opened after phase 1

        vaug = {}   # (h, bj) -> [128, 65]: cols 0..63 = v, col 64 = 1
        vaug2 = {}  # (p, bj) -> [128, 128]: v columns of head pair p

        def build_vaug(h, bj):
            va = late["vap"].tile([128, 65], F32R, name=f"va{h}_{bj}", tag=f"va{bj}",
                          bufs=2)
            nc.vector.tensor_copy(out=va[:, 0:64],
                                  in_=vtiles[bj][:, h * 64:(h + 1) * 64])
            nc.vector.tensor_copy(out=va[:, 64:65], in_=ones_col)
            vaug[(h, bj)] = va

        def build_vaug2(p, bj):
            va2 = vbp.tile([128, 128], F32R, name=f"vb{p}_{bj}", tag=f"vb{bj}")
            nc.vector.tensor_copy(out=va2,
                                  in_=vtiles[bj][:, p * 128:(p + 1) * 128])
            vaug2[(p, bj)] = va2

        mc2sb = {}   # p -> [128, T] staged MC(+alpha*I) for the head pair
        mcodd = {}   # p -> [64, T] odd head's half realigned to partitions 0-63

        def emit_pair_mc(p):
            if not (have_mc or have_ai):
                return
            for bj in range(NB):
                build_vaug2(p, bj)
            sb = mcstage.tile([128, T], F32, name=f"mc2sb{p}", tag="mc2sb", bufs=1)
            for ct in range(4):
                c0 = ct * 512
                hi = c0 + 512
                mcc = mccp.tile([128, 512], F32, name="mcc", tag="mcc", bufs=1)
                last_diag = not have_mc
                for bj in range(ct * 4, ct * 4 + 4):
                    nc.tensor.matmul(
                        mcc[:, bj * 128 - c0:bj * 128 - c0 + 128],
                        r(vaug2[(p, bj)], "mc_r"),
                        r(mcdiag[bj], "mc_r"),
                        start=(bj == ct * 4),
                        stop=(last_diag and bj == ct * 4 + 3),
                    )
                if have_mc:
                    for bj in range(ct * 4 + 1, NB):
                        hi2 = min(bj * 128, hi)
                        nc.tensor.matmul(
                            mcc[:, 0:hi2 - c0],
                            r(vaug2[(p, bj)], "mc_r"),
                            r(mcb[:, c0:hi2], "mc_r"),
                            start=False, stop=(bj == NB - 1),
                        )
                pcopy(sb[:, c0:hi], mcc, ct)
            mc2sb[p] = sb
            # realign the odd head's half to partitions 0-63
            mo = mcstage.tile([64, T], F32, name=f"mcodd{p}", tag="mcodd", bufs=1)
            nc.sync.dma_start(out=mo, in_=sb[64:128, :])
            mcodd[p] = mo

        def mc_slice(h, c0, hi):
            if h % 2 == 0:
                return mc2sb[h // 2][0:64, c0:hi]
            return mcodd[h // 2][:, c0:hi]

        def emit_chunk(h, ct, yTout):
            qTh = qkT[h // 2][(h % 2) * 64:(h % 2) * 64 + 64, :]
            kTh = qkT[2 + h // 2][(h % 2) * 64:(h % 2) * 64 + 64, :]
            c0 = ct * 512
            hi = c0 + 512
            # --- ST -> exp -> PV accumulation for this 512-wide chunk ---
            yTc = accp.tile([65, 512], F32, name="yTc", tag="yTc")
            for bj in range(ct * 4 + 4):
                lo = max(bj * 128, c0)
                n = hi - lo
                st = stp.tile([128, 512], F32, name="st", tag="st")
                nc.tensor.matmul(
                    st[:, 0:n],
                    r(kTh[:, bj * 128:(bj + 1) * 128], "st_r"),
                    r(qTh[:, lo:hi], "st_r"),
                    start=True, stop=True,
                )
                if lo == bj * 128:
                    # causal mask on the diagonal block: -1e30 where j > i
                    nc.vector.tensor_add(st[:, 0:128], st[:, 0:128], negmask)
                ex = late["expool"].tile([128, 512], F32R, name="ex", tag="ex", bufs=4)
                nc.scalar.activation(out=ex[:, 0:n], in_=st[:, 0:n],
                                     func=EXP, scale=0.125)
                nc.tensor.matmul(
                    yTc[:, lo - c0:512],
                    r(vaug[(h, bj)], "pv_r"),
                    r(ex[:, 0:n], "pv_r"),
                    start=(bj == 0), stop=(bj == ct * 4 + 3),
                )

            # --- softmax normalization: yTout = yTc[0:64] * (beta/sums) ---
            sums = late["srp"].tile([1, 512], F32, name="sums", tag="sums")
            nc.vector.tensor_copy(out=sums, in_=yTc[64:65, :])
            recip = late["srp"].tile([1, 512], F32, name="recip", tag="recip")
            nc.vector.reciprocal(out=recip, in_=sums)
            # fp32r broadcast with residual compensation: bc = beta*(hi + lo)
            recipr = late["srp"].tile([1, 512], F32R, name="recipr", tag="recipr")
            nc.vector.tensor_copy(out=recipr, in_=recip)
            rlo = late["srp"].tile([1, 512], F32, name="rlo", tag="rlo")
            nc.vector.tensor_sub(rlo, recip, recipr.bitcast(F32))
            rlor = late["srp"].tile([1, 512], F32R, name="rlor", tag="rlor")
            nc.vector.tensor_copy(out=rlor, in_=rlo)
            bc = bcp.tile([64, 512], F32, name="bc", tag="bc")
            nc.tensor.matmul(bc, r(brow, "bc_r"), r(recipr, "bc_r"),
                             start=True, stop=False)
            nc.tensor.matmul(bc, r(brow, "bc_r"), r(rlor, "bc_r"),
                             start=False, stop=True)
            bcs = late["expool"].tile([64, 512], F32, name="bcs", tag="bcs", bufs=2)
            nc.vector.tensor_copy(out=bcs, in_=bc)
            nc.vector.tensor_mul(yTout[:, c0:hi], yTc[0:64, :], bcs)

            # --- MC correction + alpha*I for this chunk (pair-staged) ---
            if have_mc or have_ai:
                nc.vector.tensor_add(yTout[:, c0:hi], yTout[:, c0:hi],
                                     mc_slice(h, c0, hi))

            if ct == 3 and have_mc:
                # dense last row of MC: y[T-1] -= gamma/T * colsum(v), with
                # colsum recovered from mc[:,0] = -g/(T-1)*(colsum - v0) + a*v0
                c1 = -(gamma + (T - 1) * alpha) / T
                c2 = (T - 1) / float(T)
                sl2 = yTout[:, T - 1:T]
                nc.vector.scalar_tensor_tensor(
                    out=sl2, in0=v0col[h], scalar=c1, in1=sl2,
                    op0=OP.mult, op1=OP.add)
                nc.vector.scalar_tensor_tensor(
                    out=sl2, in0=mc_slice(h, 0, 1), scalar=c2, in1=sl2,
                    op0=OP.mult, op1=OP.add)

            # transpose this chunk back to natural layout; DMA per pair
            for bi in range(ct * 4, ct * 4 + 4):
                ot = otp.tile([128, 64], F32, name="ot", tag="ot")
                nc.tensor.transpose(ot, yTout[:, bi * 128:(bi + 1) * 128],
                                    ident[0:64, 0:64])
                pcopy(ysb2[bi][:, (h % 2) * 64:(h % 2) * 64 + 64], ot, bi + h)
                if h % 2 == 1:
                    p = h // 2
                    nc.sync.dma_start(
                        out=y[bi * 128:(bi + 1) * 128, p * 128:(p + 1) * 128],
                        in_=ysb2[bi])

        ysb2 = [consts.tile([128, 128], F32, name=f"ysb{bi}", tag=f"ysb{bi}")
                for bi in range(NB)]

        # ---- phase 1: DMAs + projection; PE also runs pair-0 MC as filler ----
        with ExitStack() as s1:
            xTp = s1.enter_context(tc.tile_pool(name="xTp", bufs=1))
            wqp = s1.enter_context(tc.tile_pool(name="wqp", bufs=1))

            xT = [xTp.tile([128, T], F32R, name=f"xT{cc}", tag=f"xT{cc}")
                  for cc in range(KC)]
            wqkT = [wqp.tile([128, 4 * 128], F32R,
                             name=f"wqkT{cc}", tag=f"wqkT{cc}")
                    for cc in range(KC)]
            def dma_xt(nt):
                for cc in range(KC):
                    nc.sync.dma_start(
                        out=xT[cc][:, nt * 512:(nt + 1) * 512],
                        in_=xt[cc * 128:(cc + 1) * 128,
                               nt * 512:(nt + 1) * 512].bitcast(F32R))

            for cc in range(KC):
                nc.sync.dma_start(out=wqkT[cc],
                                  in_=wt[cc * 128:(cc + 1) * 128, :].bitcast(F32R))
            dma_xt(0)
            for bt in range(NB):
                nc.sync.dma_start(out=vtiles[bt],
                                  in_=xv[bt * 128:(bt + 1) * 128, :])
            for nt in range(1, 4):
                dma_xt(nt)

            ci = 0

            def emit_proj(nt):
                nonlocal ci
                for mt in (0, 2, 1, 3):
                    pp = stp.tile([128, 512], F32, name="pp", tag="st")
                    for cc in range(KC):
                        nc.tensor.matmul(
                            pp,
                            r(wqkT[cc][:, mt * 128:(mt + 1) * 128], "proj_r"),
                            r(xT[cc][:, nt * 512:(nt + 1) * 512], "proj_r"),
                            start=(cc == 0), stop=(cc == KC - 1),
                        )
                    pcopy(qkT[mt][:, nt * 512:(nt + 1) * 512], pp, ci)
                    ci += 1

            emit_proj(0)
            emit_pair_mc(0)  # PE filler while xT nt=1..3 stream in
            for nt in range(1, 4):
                emit_proj(nt)

        # ---- attention-phase SBUF pools (xT/wqkT space now free) ----
        late["expool"] = ctx.enter_context(tc.tile_pool(name="expool", bufs=3))
        late["vap"] = ctx.enter_context(tc.tile_pool(name="vap", bufs=1))
        late["srp"] = ctx.enter_context(tc.tile_pool(name="srp", bufs=1))
        late["outp"] = ctx.enter_context(tc.tile_pool(name="outp", bufs=2))

        # ---- attention heads ----
        for h in range(HL):
            for bj in range(NB):
                build_vaug(h, bj)
            if h == 2:
                emit_pair_mc(1)
            yTout = late["outp"].tile([64, T], F32, name=f"yTout{h}", tag="yTout")
            for ct in range(4):
                emit_chunk(h, ct, yTout)


_BUILD_CACHE = {}


def build_nc(alpha, beta, gamma, cfg=None):
    cfg = dict(CFG if cfg is None else cfg)
    key = (alpha, beta, gamma, tuple(sorted(cfg.items())))
    if key in _BUILD_CACHE:
        return _BUILD_CACHE[key]
    nc = bacc.Bacc("TRN2", target_bir_lowering=False, debug=False,
                   num_devices=NCORES)
    xt = nc.dram_tensor("xt", [C, T], F32, kind="ExternalInput").ap()
    xv = nc.dram_tensor("xv", [T, GC], F32, kind="ExternalInput").ap()
    wt = nc.dram_tensor("wt", [C, 2 * GC], F32, kind="ExternalInput").ap()
    y = nc.dram_tensor("y", [T, GC], F32, kind="ExternalOutput").ap()
    with tile.TileContext(nc) as tc:
        _emit(tc, xt, xv, wt, y, alpha, beta, gamma, cfg)
    nc.compile()
    _BUILD_CACHE[key] = nc
    return nc


def make_in_maps(x, w):
    xts = [np.ascontiguousarray(x[b].T) for b in range(B)]
    in_maps = []
    for c in range(NCORES):
        b, g = c // HL, c % HL
        wqk = np.concatenate(
            [w[GC * g:GC * (g + 1)], w[C + GC * g:C + GC * (g + 1)]], axis=0)
        in_maps.append({
            "xt": xts[b],
            "xv": np.ascontiguousarray(x[b][:, GC * g:GC * (g + 1)]),
            "wt": np.ascontiguousarray(wqk.T),
        })
    return in_maps


def kernel(x, w_attn, alpha, beta, gamma, n_head, **run_kwargs):
    global LAST_RESULTS
    x = np.asarray(x, dtype=np.float32)
    w = np.asarray(w_attn, dtype=np.float32)
    assert int(n_head) == H and x.shape == (B, T, C)
    nc = build_nc(float(alpha), float(beta), float(gamma))
    res = run_bass_kernel_spmd(nc, make_in_maps(x, w), list(range(NCORES)),
                               **run_kwargs)
    LAST_RESULTS = res
    out = np.empty((B, T, C), dtype=np.float32)
    for c in range(NCORES):
        b, g = c // HL, c % HL
        out[b][:, GC * g:GC * (g + 1)] = res.results[c]["y"]
    return out


# revision 32
# speedup vs baseline: 1.0662x; 1.0662x over previous
"""CausalShapedAttention Trainium2 Bass kernel.

Problem: y = (beta*softmax(causal(q k^T / sqrt(D))) + alpha*I - gamma*MC) @ v
  with qk = x @ w_attn^T (q,k halves), v = x reshaped; B=2, T=2048, C=1024, H=16, D=64.
  MC[i,j] = 1/(T-1-i) for j>i (i<T-1); MC[T-1,:] = 1/T.

Sharding: 8 cores; core c -> batch b=c//4, head-group g=c%4 (4 heads each).
Each core is fully independent (no collectives). The host passes x^T and the
per-core W-slice^T so no on-chip transposes of the inputs are needed.

Per-core dataflow (transposed-S formulation, no attention-matrix transposes):
  qT,kT = W^T-chunks^T @ xT-chunks (PSUM accum)          [64, T] per head
  per head, per 512-wide query chunk ct:
    for key-block bj (128 rows of k, causal bj*128 <= chunk end):
      ST[j,i] = kT_bj . qT-chunk   (PE, fp32r)
      exp via ACT (fused 1/sqrt(D) scale) -> SBUF fp32r
      yTc[0:65, i] += [v_bj | 1]^T @ expST   (row 64 = softmax sums)
    normalize: yTout-chunk = yTc[0:64] * (beta/sums)  (ones-matmul broadcast)
    mc-chunk: dense column-constant-tile matmuls add -gamma*MC@v + alpha*v
  the dense last row of MC is patched analytically from mc[:,0] (closed form);
  yTout is PE-transposed back to natural [T, D] layout and DMA'd out.
"""
import sys

for _p in ("/opt/trn_rl_repo",):
    if _p not in sys.path:
        sys.path.insert(0, _p)

from contextlib import ExitStack

import numpy as np

import concourse.bass as bass
import concourse.tile as tile
from concourse import bacc, mybir
from concourse.bass_utils import run_bass_kernel_spmd

F32 = mybir.dt.float32
F32R = mybir.dt.float32r
EXP = mybir.ActivationFunctionType.Exp
OP = mybir.AluOpType

B, T, C, H, D = 2, 2048, 1024, 16, 64
HL = 4            # heads per core
GC = HL * D       # channels per head-group (256)
NCORES = 8
NB = T // 128     # 16 key/query row blocks
KC = C // 128     # 8 contraction chunks

# matmul dtype knobs (float32r = full-rate PE mode, fp32 = exact but 4 cyc/row)
CFG = dict(proj_r=True, st_r=True, pv_r=True, bc_r=True, mc_r=True)

LAST_RESULTS = None  # BassKernelResults of the most recent run (for test.py)


def _emit(tc: tile.TileContext, xt, xv, wt, y, alpha, beta, gamma, cfg):
    nc = tc.nc

    def r(ap, knob):
        return ap.bitcast(F32R) if cfg[knob] else ap.bitcast(F32)

    have_mc = gamma != 0.0
    have_ai = alpha != 0.0

    with ExitStack() as ctx:
        ctx.enter_context(nc.allow_low_precision(
            reason="float32r operands for full-rate PE matmuls"))
        consts = ctx.enter_context(tc.tile_pool(name="consts", bufs=1))

        # identity for PE transposes
        ident = consts.tile([128, 128], F32, name="ident", tag="ident")
        nc.vector.memset(ident, 1.0)
        nc.gpsimd.affine_select(
            out=ident, in_=ident, compare_op=OP.is_equal, fill=0.0,
            base=0, pattern=[[-1, 128]], channel_multiplier=1,
        )

        # beta row for the sums-broadcast matmul (rounded to fp32r)
        brow_f = consts.tile([1, 64], F32, name="brow_f", tag="brow_f")
        nc.vector.memset(brow_f, beta)
        brow = consts.tile([1, 64], F32R, name="brow", tag="brow")
        nc.vector.tensor_copy(out=brow, in_=brow_f)

        # additive causal mask for diagonal ST blocks, applied on the PE:
        # st += negmaskT.T @ I with negmaskT[i,j] = -1e30 where j > i
        negmaskT_f = consts.tile([128, 128], F32, name="negmaskT_f",
                                 tag="negmaskT_f")
        nc.vector.memset(negmaskT_f, 0.0)
        nc.gpsimd.affine_select(
            out=negmaskT_f, in_=negmaskT_f, compare_op=OP.is_ge, fill=-1e30,
            base=0, pattern=[[-1, 128]], channel_multiplier=1,
        )
        BF16 = mybir.dt.bfloat16
        negmaskT = consts.tile([128, 128], BF16, name="negmaskT", tag="negmaskT")
        nc.vector.tensor_copy(out=negmaskT, in_=negmaskT_f)
        identr = consts.tile([128, 128], BF16, name="identr", tag="identr")
        nc.vector.tensor_copy(out=identr, in_=ident)
        ones_col = consts.tile([128, 1], F32, name="ones_col", tag="ones_col")
        nc.vector.memset(ones_col, 1.0)

        # v[0, :] per head as a [64,1] column (for the dense-last-row patch)
        v0col = [consts.tile([64, 1], F32, name=f"v0c{h}", tag=f"v0c{h}")
                 for h in range(HL)]
        for h in range(HL):
            nc.sync.dma_start(
                out=v0col[h],
                in_=xv[0:1, h * 64:(h + 1) * 64].rearrange("a b -> b a"))

        # persistent SBUF tensors
        qkT = [consts.tile([128, T], F32R, name=f"qkT{mt}", tag=f"qkT{mt}")
               for mt in range(4)]
        vtiles = [consts.tile([128, GC], F32, name=f"v{bt}", tag=f"v{bt}")
                  for bt in range(NB)]

        def pcopy(dst, src, i):
            # alternate PSUM->SBUF copies across DVE/ACT
            if i % 2 == 0:
                nc.vector.tensor_copy(out=dst, in_=src)
            else:
                nc.scalar.copy(out=dst, in_=src)

        # ---- mc / alpha*I constant tiles (DVE/Pool work, fills DMA window) ----
        mcb = None
        mcdiag = []
        with ExitStack() as s2:
            scratch = s2.enter_context(tc.tile_pool(name="mcscratch", bufs=1))
            aI_f = None
            if have_ai:
                aI_f = scratch.tile([128, 128], F32, name="aI_f", tag="aI_f")
                nc.vector.memset(aI_f, alpha)
                nc.gpsimd.affine_select(
                    out=aI_f, in_=aI_f, compare_op=OP.is_equal, fill=0.0,
                    base=0, pattern=[[-1, 128]], channel_multiplier=1,
                )
            if have_mc:
                mcs = scratch.tile([128, T], F32, name="mcs", tag="mcs")
                nc.gpsimd.iota(mcs, pattern=[[-1, T]], base=T - 1,
                               channel_multiplier=0,
                               allow_small_or_imprecise_dtypes=True)
                nc.vector.memset(mcs[:, T - 1:T], 1.0)  # avoid 1/0; fixed below
                mcf = scratch.tile([128, T], F32, name="mcf", tag="mcf")
                nc.vector.reciprocal(out=mcf, in_=mcs)
                nc.vector.tensor_scalar_mul(mcf, mcf, -gamma)
                nc.vector.memset(mcf[:, T - 1:T], 0.0)
                mcb = consts.tile([128, T], F32R, name="mcb", tag="mcb")
                nc.vector.tensor_copy(out=mcb, in_=mcf)
                mdf = scratch.tile([128, 128], F32, name="mdf", tag="mdf")
                for bj in range(NB):
                    # keep strictly-lower (j > i) of the column-constant strip
                    nc.gpsimd.affine_select(
                        out=mdf, in_=mcf[:, bj * 128:(bj + 1) * 128],
                        compare_op=OP.is_gt, fill=0.0,
                        base=0, pattern=[[-1, 128]], channel_multiplier=1,
                    )
                    if have_ai:
                        nc.vector.tensor_add(mdf, mdf, aI_f)
                    md = consts.tile([128, 128], F32R, name=f"mcd{bj}",
                                     tag=f"mcd{bj}")
                    nc.vector.tensor_copy(out=md, in_=mdf)
                    mcdiag.append(md)
            elif have_ai:
                aI = consts.tile([128, 128], F32R, name="aI", tag="aI")
                nc.vector.tensor_copy(out=aI, in_=aI_f)
                mcdiag = [aI] * NB

        # ---- PSUM pools (2+2+1+1+2 = 8 banks; proj shares the st slots) ----
        stp = ctx.enter_context(tc.tile_pool(name="stp", bufs=3, space="PSUM"))
        accp = ctx.enter_context(tc.tile_pool(name="accp", bufs=2, space="PSUM"))
        mccp = stp  # mcc tag shares the stp pool (1 extra bank)
        bcp = ctx.enter_context(tc.tile_pool(name="bcp", bufs=1, space="PSUM"))
        otp = ctx.enter_context(tc.tile_pool(name="otp", bufs=1, space="PSUM"))
        vbp = ctx.enter_context(tc.tile_pool(name="vbp", bufs=1))
        mcstage = ctx.enter_context(tc.tile_pool(name="mcstage", bufs=1))
        late = {}  # attention-phase SBUF pools, opened after phase 1

        vaug = {}   # (h, bj) -> [128, 65]: cols 0..63 = v, col 64 = 1
        vaug2 = {}  # (p, bj) -> [128, 128]: v columns of head pair p

        def build_vaug(h, bj):
            va = late["vap"].tile([128, 65], F32R, name=f"va{h}_{bj}", tag=f"va{bj}",
                          bufs=2)
            nc.vector.tensor_copy(out=va[:, 0:64],
                                  in_=vtiles[bj][:, h * 64:(h + 1) * 64])
            nc.vector.tensor_copy(out=va[:, 64:65], in_=ones_col)
            vaug[(h, bj)] = va

        def build_vaug2(p, bj):
            va2 = vbp.tile([128, 128], F32R, name=f"vb{p}_{bj}", tag=f"vb{bj}")
            nc.vector.tensor_copy(out=va2,
                                  in_=vtiles[bj][:, p * 128:(p + 1) * 128])
            vaug2[(p, bj)] = va2

        mc2sb = {}   # p -> [128, T] staged MC(+alpha*I) for the head pair
        mcodd = {}   # p -> [64, T] odd head's half realigned to partitions 0-63

        def emit_pair_mc(p):
            if not (have_mc or have_ai):
                return
            for bj in range(NB):
                build_vaug2(p, bj)
            sb = mcstage.tile([128, T], F32, name=f"mc2sb{p}", tag="mc2sb", bufs=1)
            for ct in range(4):
                c0 = ct * 512
                hi = c0 + 512
                mcc = mccp.tile([128, 512], F32, name="mcc", tag="mcc", bufs=1)
                last_diag = not have_mc
                for bj in range(ct * 4, ct * 4 + 4):
                    nc.tensor.matmul(
                        mcc[:, bj * 128 - c0:bj * 128 - c0 + 128],
                        r(vaug2[(p, bj)], "mc_r"),
                        r(mcdiag[bj], "mc_r"),
                        start=(bj == ct * 4),
                        stop=(last_diag and bj == ct * 4 + 3),
                    )
                if have_mc:
                    for bj in range(ct * 4 + 1, NB):
                        hi2 = min(bj * 128, hi)
                        nc.tensor.matmul(
                            mcc[:, 0:hi2 - c0],
                            r(vaug2[(p, bj)], "mc_r"),
                            r(mcb[:, c0:hi2], "mc_r"),
                            start=False, stop=(bj == NB - 1),
                        )
                pcopy(sb[:, c0:hi], mcc, ct)
            mc2sb[p] = sb
            # realign the odd head's half to partitions 0-63
            mo = mcstage.tile([64, T], F32, name=f"mcodd{p}", tag="mcodd", bufs=1)
            nc.sync.dma_start(out=mo, in_=sb[64:128, :])
            mcodd[p] = mo

        def mc_slice(h, c0, hi):
            if h % 2 == 0:
                return mc2sb[h // 2][0:64, c0:hi]
            return mcodd[h // 2][:, c0:hi]

        def emit_chunk(h, ct, yTout):
            qTh = qkT[h // 2][(h % 2) * 64:(h % 2) * 64 + 64, :]
            kTh = qkT[2 + h // 2][(h % 2) * 64:(h % 2) * 64 + 64, :]
            c0 = ct * 512
            hi = c0 + 512
            # --- ST -> exp -> PV accumulation for this 512-wide chunk ---
            yTc = accp.tile([65, 512], F32, name="yTc", tag="yTc")
            for bj in range(ct * 4 + 4):
                lo = max(bj * 128, c0)
                n = hi - lo
                st = stp.tile([128, 512], F32, name="st", tag="st")
                diag = lo == bj * 128
                nc.tensor.matmul(
                    st[:, 0:n],
                    r(kTh[:, bj * 128:(bj + 1) * 128], "st_r"),
                    r(qTh[:, lo:hi], "st_r"),
                    start=True, stop=not diag,
                )
                if diag:
                    # causal mask: accumulate -1e30 strict-lower via the PE
                    nc.tensor.matmul(
                        st[:, 0:128], negmaskT, identr,
                        start=False, stop=True,
                    )
                ex = late["expool"].tile([128, 512], F32R, name="ex", tag="ex", bufs=4)
                nc.scalar.activation(out=ex[:, 0:n], in_=st[:, 0:n],
                                     func=EXP, scale=0.125)
                nc.tensor.matmul(
                    yTc[:, lo - c0:512],
                    r(vaug[(h, bj)], "pv_r"),
                    r(ex[:, 0:n], "pv_r"),
                    start=(bj == 0), stop=(bj == ct * 4 + 3),
                )

            # --- softmax normalization: yTout = yTc[0:64] * (beta/sums) ---
            sums = late["srp"].tile([1, 512], F32, name="sums", tag="sums")
            nc.vector.tensor_copy(out=sums, in_=yTc[64:65, :])
            recip = late["srp"].tile([1, 512], F32, name="recip", tag="recip")
            nc.vector.reciprocal(out=recip, in_=sums)
            # fp32r broadcast with residual compensation: bc = beta*(hi + lo)
            recipr = late["srp"].tile([1, 512], F32R, name="recipr", tag="recipr")
            nc.vector.tensor_copy(out=recipr, in_=recip)
            rlo = late["srp"].tile([1, 512], F32, name="rlo", tag="rlo")
            nc.vector.tensor_sub(rlo, recip, recipr.bitcast(F32))
            rlor = late["srp"].tile([1, 512], F32R, name="rlor", tag="rlor")
            nc.vector.tensor_copy(out=rlor, in_=rlo)
            bc = bcp.tile([64, 512], F32, name="bc", tag="bc")
            nc.tensor.matmul(bc, r(brow, "bc_r"), r(recipr, "bc_r"),
                             start=True, stop=False)
            nc.tensor.matmul(bc, r(brow, "bc_r"), r(rlor, "bc_r"),
                             start=False, stop=True)
            bcs = late["expool"].tile([64, 512], F32, name="bcs", tag="bcs", bufs=2)
            nc.vector.tensor_copy(out=bcs, in_=bc)
            nc.vector.tensor_mul(yTout[:, c0:hi], yTc[0:64, :], bcs)

            # --- MC correction + alpha*I for this chunk (pair-staged) ---
            if have_mc or have_ai:
                nc.vector.tensor_add(yTout[:, c0:hi], yTout[:, c0:hi],
                                     mc_slice(h, c0, hi))

            if ct == 3 and have_mc:
                # dense last row of MC: y[T-1] -= gamma/T * colsum(v), with
                # colsum recovered from mc[:,0] = -g/(T-1)*(colsum - v0) + a*v0
                c1 = -(gamma + (T - 1) * alpha) / T
                c2 = (T - 1) / float(T)
                sl2 = yTout[:, T - 1:T]
                nc.vector.scalar_tensor_tensor(
                    out=sl2, in0=v0col[h], scalar=c1, in1=sl2,
                    op0=OP.mult, op1=OP.add)
                nc.vector.scalar_tensor_tensor(
                    out=sl2, in0=mc_slice(h, 0, 1), scalar=c2, in1=sl2,
                    op0=OP.mult, op1=OP.add)

            # transpose this chunk back to natural layout; DMA per pair
            for bi in range(ct * 4, ct * 4 + 4):
                ot = otp.tile([128, 64], F32, name="ot", tag="ot")
                nc.tensor.transpose(ot, yTout[:, bi * 128:(bi + 1) * 128],
                                    ident[0:64, 0:64])
                pcopy(ysb2[bi][:, (h % 2) * 64:(h % 2) * 64 + 64], ot, bi + h)
                if h % 2 == 1:
                    p = h // 2
                    nc.sync.dma_start(
                        out=y[bi * 128:(bi + 1) * 128, p * 128:(p + 1) * 128],
                        in_=ysb2[bi])

        ysb2 = [consts.tile([128, 128], F32, name=f"ysb{bi}", tag=f"ysb{bi}")
                for bi in range(NB)]

        # ---- phase 1: DMAs + projection; PE also runs pair-0 MC as filler ----
        with ExitStack() as s1:
            xTp = s1.enter_context(tc.tile_pool(name="xTp", bufs=1))
            wqp = s1.enter_context(tc.tile_pool(name="wqp", bufs=1))

            xT = [xTp.tile([128, T], F32R, name=f"xT{cc}", tag=f"xT{cc}")
                  for cc in range(KC)]
            wqkT = [wqp.tile([128, 4 * 128], F32R,
                             name=f"wqkT{cc}", tag=f"wqkT{cc}")
                    for cc in range(KC)]
            def dma_xt(nt):
                for cc in range(KC):
                    nc.sync.dma_start(
                        out=xT[cc][:, nt * 512:(nt + 1) * 512],
                        in_=xt[cc * 128:(cc + 1) * 128,
                               nt * 512:(nt + 1) * 512].bitcast(F32R))

            for cc in range(KC):
                nc.sync.dma_start(out=wqkT[cc],
                                  in_=wt[cc * 128:(cc + 1) * 128, :].bitcast(F32R))
            dma_xt(0)
            for bt in range(NB):
                nc.sync.dma_start(out=vtiles[bt],
                                  in_=xv[bt * 128:(bt + 1) * 128, :])
            for nt in range(1, 4):
                dma_xt(nt)

            ci = 0

            def emit_proj(nt):
                nonlocal ci
                for mt in (0, 2, 1, 3):
                    pp = stp.tile([128, 512], F32, name="pp", tag="st")
                    for cc in range(KC):
                        nc.tensor.matmul(
                            pp,
                            r(wqkT[cc][:, mt * 128:(mt + 1) * 128], "proj_r"),
                            r(xT[cc][:, nt * 512:(nt + 1) * 512], "proj_r"),
                            start=(cc == 0), stop=(cc == KC - 1),
                        )
                    pcopy(qkT[mt][:, nt * 512:(nt + 1) * 512], pp, ci)
                    ci += 1

            emit_proj(0)
            emit_pair_mc(0)  # PE filler while xT nt=1..3 stream in
            for nt in range(1, 4):
                emit_proj(nt)

        # ---- attention-phase SBUF pools (xT/wqkT space now free) ----
        late["expool"] = ctx.enter_context(tc.tile_pool(name="expool", bufs=3))
        late["vap"] = ctx.enter_context(tc.tile_pool(name="vap", bufs=1))
        late["srp"] = ctx.enter_context(tc.tile_pool(name="srp", bufs=1))
        late["outp"] = ctx.enter_context(tc.tile_pool(name="outp", bufs=2))

        # ---- attention heads ----
        for h in range(HL):
            for bj in range(NB):
                build_vaug(h, bj)
            if h == 2:
                emit_pair_mc(1)
            yTout = late["outp"].tile([64, T], F32, name=f"yTout{h}", tag="yTout")
            for ct in range(4):
                emit_chunk(h, ct, yTout)


_BUILD_CACHE = {}


def build_nc(alpha, beta, gamma, cfg=None):
    cfg = dict(CFG if cfg is None else cfg)
    key = (alpha, beta, gamma, tuple(sorted(cfg.items())))
    if key in _BUILD_CACHE:
        return _BUILD_CACHE[key]
    nc = bacc.Bacc("TRN2", target_bir_lowering=False, debug=False,
                   num_devices=NCORES)
    xt = nc.dram_tensor("xt", [C, T], F32, kind="ExternalInput").ap()
    xv = nc.dram_tensor("xv", [T, GC], F32, kind="ExternalInput").ap()
    wt = nc.dram_tensor("wt", [C, 2 * GC], F32, kind="ExternalInput").ap()
    y = nc.dram_tensor("y", [T, GC], F32, kind="ExternalOutput").ap()
    with tile.TileContext(nc) as tc:
        _emit(tc, xt, xv, wt, y, alpha, beta, gamma, cfg)
    nc.compile()
    _BUILD_CACHE[key] = nc
    return nc


def make_in_maps(x, w):
    xts = [np.ascontiguousarray(x[b].T) for b in range(B)]
    in_maps = []
    for c in range(NCORES):
        b, g = c // HL, c % HL
        wqk = np.concatenate(
            [w[GC * g:GC * (g + 1)], w[C + GC * g:C + GC * (g + 1)]], axis=0)
        in_maps.append({
            "xt": xts[b],
            "xv": np.ascontiguousarray(x[b][:, GC * g:GC * (g + 1)]),
            "wt": np.ascontiguousarray(wqk.T),
        })
    return in_maps


def kernel(x, w_attn, alpha, beta, gamma, n_head, **run_kwargs):
    global LAST_RESULTS
    x = np.asarray(x, dtype=np.float32)
    w = np.asarray(w_attn, dtype=np.float32)
    assert int(n_head) == H and x.shape == (B, T, C)
    nc = build_nc(float(alpha), float(beta), float(gamma))
    res = run_bass_kernel_spmd(nc, make_in_maps(x, w), list(range(NCORES)),
                               **run_kwargs)
    LAST_RESULTS = res
    out = np.empty((B, T, C), dtype=np.float32)
    for c in range(NCORES):
        b, g = c // HL, c % HL
        out[b][:, GC * g:GC * (g + 1)] = res.results[c]["y"]
    return out
